# revision 1
# baseline (speedup 1.0000x reference)
"""GQA attention block (QKV proj + causal attention + output proj) on 8 trn2 cores.

Sharding: core c -> (batch b = c//4, kv-group g = c%4). Each core computes 4 Q
heads (one KV-head group) of one batch and a partial o_proj output; the host
sums the 4 partials per batch (row-sharded o_proj all-reduce done host-side).

All device matmuls are fp32 (exact). Attention uses transposed scores
S^T[tk, tq] so the softmax denominator comes for free from a ones-column
appended to V, and no on-chip transposes of attention weights are needed.
"""

import math

import numpy as np

# Model dims (hardcoded per contract; kernel.py must be self-contained).
B = 2
T = 2048
E = 2048
HD = 128               # head dim
NH = 16                # query heads total
NKV = 4                # kv heads total
NHC = 4                # query heads per core
P = 128
KO = E // P            # 16 contraction subtiles of 128
NQUART = 4             # xT streamed in 4 e-quarters of 512
TQC = T // 512         # 4 query chunks of 512
TB = T // P            # 16 t blocks of 128
SCALE = 1.0 / math.sqrt(HD)
N_CORES = 8

_NC_CACHE = {}


def _build_nc():
    import concourse.bacc as bacc
    import concourse.mybir as mybir
    import concourse.tile as tile
    from concourse.masks import make_identity, make_upper_triangular

    f32 = mybir.dt.float32
    nc = bacc.Bacc(None, target_bir_lowering=False)

    xT = nc.dram_tensor("xT", [E, T], f32, kind="ExternalInput")
    wqT = nc.dram_tensor("wqT", [E, NHC * HD], f32, kind="ExternalInput")
    wkT = nc.dram_tensor("wkT", [E, HD], f32, kind="ExternalInput")
    wvT = nc.dram_tensor("wvT", [E, HD], f32, kind="ExternalInput")
    woT = nc.dram_tensor("woT", [NHC * HD, E], f32, kind="ExternalInput")
    out = nc.dram_tensor("out", [T, E], f32, kind="ExternalOutput")

    xT_r = xT.rearrange("(ko p) t -> p ko t", p=P)        # [128, 16, T]
    wqT_r = wqT.rearrange("(ko p) d -> p ko d", p=P)      # [128, 16, 512]
    wkT_r = wkT.rearrange("(ko p) d -> p ko d", p=P)      # [128, 16, 128]
    wvT_r = wvT.rearrange("(ko p) d -> p ko d", p=P)
    woT_r = woT.rearrange("(h p) e -> p h e", p=P)        # [128, 4, E]
    out_r = out.rearrange("(tb p) e -> p tb e", p=P)      # [128, 16, E]

    with tile.TileContext(nc) as tc:
        with (
            tc.tile_pool(name="const", bufs=1) as constp,
            tc.tile_pool(name="qkv", bufs=1) as qkvp,
            tc.tile_pool(name="ps_acc", bufs=2, space="PSUM") as ps_acc,
            tc.tile_pool(name="ps_y", bufs=4, space="PSUM") as ps_y,
            tc.tile_pool(name="ps_t", bufs=2, space="PSUM") as ps_t,
        ):
            identity = constp.tile([P, P], f32, tag="ident")
            make_identity(nc, identity)

            # tri[p, q] = 1.0 where p <= q — causal mask for the one
            # tk==tq diagonal 128x128 sub-block.
            tri = constp.tile([P, P], f32, tag="tri")
            make_upper_triangular(nc, tri[:], val=1.0, diag=True)

            QT = qkvp.tile([P, NHC, T], f32, tag="QT")     # q^T per head [d, t]
            KT = qkvp.tile([P, T], f32, tag="KT")          # k^T [d, t]
            VT = qkvp.tile([P, T], f32, tag="VT")          # v^T [d, t]
            VAUG = qkvp.tile([P, TB, HD + 1], f32, tag="VAUG")  # v blocks [tk, 129]

            # ---- Phase 1: projections. q^T/k^T/v^T = W @ x^T, contracting
            # over e; xT streamed in 4 e-quarters, weights resident.
            with (
                tc.tile_pool(name="w1", bufs=1) as w1p,
                tc.tile_pool(name="xq", bufs=2) as xqp,
            ):
                WQT = w1p.tile([P, KO, NHC * HD], f32, tag="WQT")
                WKT = w1p.tile([P, KO, HD], f32, tag="WKT")
                WVT = w1p.tile([P, KO, HD], f32, tag="WVT")

                for q in range(NQUART):
                    xt = xqp.tile([P, 4, T], f32, tag="xq")
                    # Interleave activations and weights per e-subtile so the
                    # first matmuls aren't stuck behind the full weight load.
                    for eo in range(4):
                        ko = 4 * q + eo
                        nc.sync.dma_start(xt[:, eo], xT_r[:, ko])
                        nc.sync.dma_start(WQT[:, ko], wqT_r[:, ko])
                        nc.sync.dma_start(WKT[:, ko], wkT_r[:, ko])
                        nc.sync.dma_start(WVT[:, ko], wvT_r[:, ko])

                    def _acc(dst, lhsT_of_eo, tcol):
                        ps = ps_acc.tile([P, 512], f32, tag="ps_acc")
                        for eo in range(4):
                            nc.tensor.matmul(
                                ps[:],
                                lhsT_of_eo(eo),
                                xt[:, eo, tcol * 512:(tcol + 1) * 512],
                                start=(eo == 0),
                                stop=(eo == 3),
                            )
                        if q == 0:
                            nc.vector.tensor_copy(dst, ps[:])
                        else:
                            nc.vector.tensor_add(out=dst, in0=dst, in1=ps[:])

                    for h in range(NHC):
                        for tcol in range(TQC):
                            _acc(
                                QT[:, h, tcol * 512:(tcol + 1) * 512],
                                lambda eo, h=h: WQT[:, 4 * q + eo, h * HD:(h + 1) * HD],
                                tcol,
                            )
                    for tcol in range(TQC):
                        _acc(
                            KT[:, tcol * 512:(tcol + 1) * 512],
                            lambda eo: WKT[:, 4 * q + eo],
                            tcol,
                        )
                    for tcol in range(TQC):
                        _acc(
                            VT[:, tcol * 512:(tcol + 1) * 512],
                            lambda eo: WVT[:, 4 * q + eo],
                            tcol,
                        )

            # v^T -> v natural layout blocks, with ones column for the
            # softmax denominator.
            nc.vector.memset(VAUG[:, :, HD:HD + 1], 1.0)
            for tb in range(TB):
                pst = ps_t.tile([P, P], f32, tag="ps_t")
                nc.tensor.transpose(pst[:], VT[:, tb * P:(tb + 1) * P], identity[:])
                nc.vector.tensor_copy(VAUG[:, tb, 0:HD], pst[:])

            # ---- Phases 2+3 pools
            with (
                tc.tile_pool(name="big2", bufs=1) as big2,
                tc.tile_pool(name="work", bufs=4) as work,
                tc.tile_pool(name="owork", bufs=3) as owork,
            ):
                YT = big2.tile([P, NHC, T], f32, tag="YT")   # y^T per head [d, t]
                WOT = big2.tile([P, NHC, E], f32, tag="WOT")
                for ko in range(4):
                    nc.sync.dma_start(
                        WOT[:, ko], woT_r[:, ko]
                    )

                # ---- Phase 2: causal attention, transposed scores. For
                # diagonal-region tk blocks the score matmul is narrowed to
                # the causally-valid tq columns; only the single tk==tq
                # 128x128 sub-block needs the triangular mask.
                for h in range(NHC):
                    for tqc in range(TQC):
                        ntk = 4 * (tqc + 1)   # tk blocks up to the diagonal
                        psy = [
                            ps_y.tile([P, HD + 1], f32, tag="ps_y", name=f"psy_{j}")
                            for j in range(4)
                        ]
                        for tk in range(ntk):
                            i = tk - 4 * tqc  # >= 0 inside the diagonal region
                            off = max(0, i) * P   # local tq offset of valid cols
                            w = 512 - off
                            pss = ps_acc.tile([P, 512], f32, tag="ps_acc")
                            nc.tensor.matmul(
                                pss[:, 0:w],
                                KT[:, tk * P:(tk + 1) * P],
                                QT[:, h, tqc * 512 + off:(tqc + 1) * 512],
                                start=True,
                                stop=True,
                            )
                            es = work.tile([P, 512], f32, tag="expS")
                            nc.scalar.activation(
                                es[:, 0:w], pss[:, 0:w],
                                mybir.ActivationFunctionType.Exp,
                                scale=SCALE,
                            )
                            if i >= 0:
                                nc.vector.tensor_mul(
                                    out=es[:, 0:P], in0=es[:, 0:P], in1=tri[:]
                                )
                            for j in range(max(0, i), 4):
                                nc.tensor.matmul(
                                    psy[j][:],
                                    es[:, j * P - off:(j + 1) * P - off],
                                    VAUG[:, tk],
                                    start=(tk == 0),
                                    stop=(tk == 4 * tqc + j),
                                )
                        for j in range(4):
                            jg = 4 * tqc + j
                            recip = work.tile([P, 1], f32, tag="recip")
                            nc.vector.reciprocal(recip[:], psy[j][:, HD:HD + 1])
                            ysb = work.tile([P, P], f32, tag="ysb")
                            nc.vector.tensor_scalar_mul(ysb[:], psy[j][:, 0:HD], recip[:])
                            pst = ps_t.tile([P, P], f32, tag="ps_t")
                            nc.tensor.transpose(pst[:], ysb[:], identity[:])
                            nc.vector.tensor_copy(YT[:, h, jg * P:(jg + 1) * P], pst[:])

                # ---- Phase 3: o_proj partial: out[t, e] = sum_h y_h^T.T @ woT_h
                for tb in range(TB):
                    for ec in range(4):
                        ps = ps_acc.tile([P, 512], f32, tag="ps_acc")
                        for h in range(NHC):
                            nc.tensor.matmul(
                                ps[:],
                                YT[:, h, tb * P:(tb + 1) * P],
                                WOT[:, h, ec * 512:(ec + 1) * 512],
                                start=(h == 0),
                                stop=(h == 3),
                            )
                        osb = owork.tile([P, 512], f32, tag="osb")
                        nc.vector.tensor_copy(osb[:], ps[:])
                        nc.sync.dma_start(out_r[:, tb, ec * 512:(ec + 1) * 512], osb[:])

    nc.finalize()
    return nc


def _get_nc():
    if "nc" not in _NC_CACHE:
        _NC_CACHE["nc"] = _build_nc()
    return _NC_CACHE["nc"]


def _in_maps(x, wq, wk, wv, wo):
    xTb = [np.ascontiguousarray(x[b].T) for b in range(B)]
    wqT = [np.ascontiguousarray(wq[g * 512:(g + 1) * 512].T) for g in range(NKV)]
    wkT = [np.ascontiguousarray(wk[g * HD:(g + 1) * HD].T) for g in range(NKV)]
    wvT = [np.ascontiguousarray(wv[g * HD:(g + 1) * HD].T) for g in range(NKV)]
    woT = [np.ascontiguousarray(wo[:, g * 512:(g + 1) * 512].T) for g in range(NKV)]
    maps = []
    for c in range(N_CORES):
        b, g = divmod(c, NKV)
        maps.append({
            "xT": xTb[b],
            "wqT": wqT[g],
            "wkT": wkT[g],
            "wvT": wvT[g],
            "woT": woT[g],
        })
    return maps


def kernel(x, wq, wk, wv, wo):
    from concourse.bass_utils import run_bass_kernel_spmd

    x = np.asarray(x, dtype=np.float32)
    wq = np.asarray(wq, dtype=np.float32)
    wk = np.asarray(wk, dtype=np.float32)
    wv = np.asarray(wv, dtype=np.float32)
    wo = np.asarray(wo, dtype=np.float32)

    nc = _get_nc()
    in_maps = _in_maps(x, wq, wk, wv, wo)

    res = run_bass_kernel_spmd(nc, in_maps, core_ids=list(range(N_CORES)))

    partials = [res.results[c]["out"] for c in range(N_CORES)]
    out = np.empty((B, T, E), dtype=np.float32)
    for b in range(B):
        acc = partials[NKV * b].astype(np.float32)
        for g in range(1, NKV):
            acc = acc + partials[NKV * b + g]
        out[b] = acc
    return out



# revision 14
# speedup vs baseline: 14.9846x; 14.9846x over previous
"""GQA attention block (QKV proj + causal attention + output proj) on 8 trn2 cores.

Sharding: core c -> (batch b = c//4, kv-group g = c%4). Each core computes 4 Q
heads (one KV-head group) of one batch and a partial o_proj output; the host
sums the 4 partials per batch (row-sharded o_proj all-reduce done host-side).

All matmul operands are bf16 (fp32 PSUM accumulation) for 1-cycle/row tensor
throughput; the rel-err budget (2e-2 normalized absmax) dwarfs bf16 rounding.
Attention uses transposed scores S^T[tk, tq] so the softmax denominator comes
for free from a ones-column appended to V, and no on-chip transposes of
attention weights are needed. Scores are exp'd in 2-bank batches to amortize
the scalar engine's fixed per-instruction cost.
"""

import math

import numpy as np

# Model dims (hardcoded per contract; kernel.py must be self-contained).
B = 2
T = 2048
E = 2048
HD = 128               # head dim
NH = 16                # query heads total
NKV = 4                # kv heads total
NHC = 4                # query heads per core
P = 128
KO = E // P            # 16 contraction subtiles of 128
NQUART = 4             # xT streamed in 4 e-quarters of 512
TQC = T // 512         # 4 query chunks of 512
TB = T // P            # 16 t blocks of 128
SCALE = 1.0 / math.sqrt(HD)
N_CORES = 8

_NC_CACHE = {}


def _build_nc():
    import concourse.bacc as bacc
    import concourse.mybir as mybir
    import concourse.tile as tile
    from concourse.masks import make_identity, make_upper_triangular

    f32 = mybir.dt.float32
    bf16 = mybir.dt.bfloat16
    f16 = mybir.dt.float16
    EXP = mybir.ActivationFunctionType.Exp
    COPY = mybir.ActivationFunctionType.Copy
    nc = bacc.Bacc(None, target_bir_lowering=False)

    xT = nc.dram_tensor("xT", [E, T], bf16, kind="ExternalInput")
    wqT = nc.dram_tensor("wqT", [E, NHC * HD], bf16, kind="ExternalInput")
    wkT = nc.dram_tensor("wkT", [E, HD], bf16, kind="ExternalInput")
    wvT = nc.dram_tensor("wvT", [E, HD], bf16, kind="ExternalInput")
    woT = nc.dram_tensor("woT", [NHC * HD, E], bf16, kind="ExternalInput")
    # fp16 partials: halves the host-visible output transfer; 10-bit mantissa
    # rounding (~0.05%) is negligible vs the rel-err budget, and partial
    # magnitudes (~|out| <= ~30) sit comfortably in fp16 range.
    out = nc.dram_tensor("out", [T, E], f16, kind="ExternalOutput")

    xT_r = xT.rearrange("(ko p) t -> p ko t", p=P)        # [128, 16, T]
    wqT_r = wqT.rearrange("(ko p) d -> p ko d", p=P)      # [128, 16, 512]
    wkT_r = wkT.rearrange("(ko p) d -> p ko d", p=P)      # [128, 16, 128]
    wvT_r = wvT.rearrange("(ko p) d -> p ko d", p=P)
    woT_r = woT.rearrange("(h p) e -> p h e", p=P)        # [128, 4, E]
    out_r = out.rearrange("(tb p) e -> p tb e", p=P)      # [128, 16, E]

    with tile.TileContext(nc) as tc:
        with (
            tc.tile_pool(name="const", bufs=1) as constp,
            tc.tile_pool(name="qkv", bufs=1) as qkvp,
        ):
            identity = constp.tile([P, P], bf16, tag="ident")
            make_identity(nc, identity)

            # tri[p, q] = 1.0 where p <= q — causal mask for the one
            # tk==tq diagonal 128x128 sub-block.
            tri = constp.tile([P, P], bf16, tag="tri")
            make_upper_triangular(nc, tri[:], val=1.0, diag=True)

            QT = qkvp.tile([P, NHC, T], bf16, tag="QT")    # q^T per head [d, t]
            KT = qkvp.tile([P, T], bf16, tag="KT")         # k^T [d, t]
            VT = qkvp.tile([P, T], bf16, tag="VT")         # v^T [d, t]
            VAUG = qkvp.tile([P, TB, HD + 4], bf16, tag="VAUG")  # v blocks [tk, 129]

            # ---- Phase 1: projections. q^T/k^T/v^T = W @ x^T, contracting
            # over e; xT streamed in 4 e-quarters, weights resident. Quarter
            # partials accumulate in f32 SBUF; the last quarter also emits the
            # bf16 copy (on the scalar engine — the vector engine carries the
            # adds).
            with (
                tc.tile_pool(name="w1", bufs=1) as w1p,
                tc.tile_pool(name="acc1", bufs=1) as accp,
                tc.tile_pool(name="xq", bufs=2) as xqp,
                tc.tile_pool(name="ps1", bufs=4, space="PSUM") as ps1,
                tc.tile_pool(name="ps1t", bufs=2, space="PSUM") as ps1t,
            ):
                WQT = w1p.tile([P, KO, NHC * HD], bf16, tag="WQT")
                WKT = w1p.tile([P, KO, HD], bf16, tag="WKT")
                WVT = w1p.tile([P, KO, HD], bf16, tag="WVT")
                QTa = accp.tile([P, NHC, T], f32, tag="QTa")
                KTa = accp.tile([P, T], f32, tag="KTa")
                VTa = accp.tile([P, T], f32, tag="VTa")

                nc.vector.memset(VAUG[:, :, HD:HD + 1], 1.0)

                for q in range(NQUART):
                    xt = xqp.tile([P, 4, T], bf16, tag="xq")
                    # Interleave activations and weights per e-subtile so the
                    # first matmuls aren't stuck behind the full weight load.
                    for eo in range(4):
                        ko = 4 * q + eo
                        nc.sync.dma_start(xt[:, eo], xT_r[:, ko])
                        nc.sync.dma_start(WQT[:, ko], wqT_r[:, ko])
                        nc.sync.dma_start(WKT[:, ko], wkT_r[:, ko])
                        nc.sync.dma_start(WVT[:, ko], wvT_r[:, ko])

                    def _acc(dsta, dstb, lhsT_of_eo, tcol):
                        ps = ps1.tile([P, 512], f32, tag="ps1")
                        for eo in range(4):
                            nc.tensor.matmul(
                                ps[:],
                                lhsT_of_eo(eo),
                                xt[:, eo, tcol * 512:(tcol + 1) * 512],
                                start=(eo == 0),
                                stop=(eo == 3),
                            )
                        if q == 0:
                            nc.vector.tensor_copy(dsta, ps[:])
                        else:
                            nc.vector.tensor_add(out=dsta, in0=dsta, in1=ps[:])
                        if q == 3:
                            nc.scalar.activation(dstb, dsta, COPY)

                    for h in range(NHC):
                        for tcol in range(TQC):
                            sl = slice(tcol * 512, (tcol + 1) * 512)
                            _acc(
                                QTa[:, h, sl],
                                QT[:, h, sl],
                                lambda eo, h=h: WQT[:, 4 * q + eo, h * HD:(h + 1) * HD],
                                tcol,
                            )
                    for tcol in range(TQC):
                        sl = slice(tcol * 512, (tcol + 1) * 512)
                        _acc(KTa[:, sl], KT[:, sl],
                             lambda eo: WKT[:, 4 * q + eo], tcol)
                    for tcol in range(TQC):
                        sl = slice(tcol * 512, (tcol + 1) * 512)
                        _acc(VTa[:, sl], VT[:, sl],
                             lambda eo: WVT[:, 4 * q + eo], tcol)
                        if q == 3:
                            # v^T -> v natural layout blocks feeding the
                            # ones-augmented V operand, as soon as each VT
                            # chunk is finalized.
                            for tb in range(4 * tcol, 4 * tcol + 4):
                                pst = ps1t.tile([P, P], bf16, tag="ps1t")
                                nc.tensor.transpose(
                                    pst[:], VT[:, tb * P:(tb + 1) * P], identity[:]
                                )
                                nc.vector.tensor_copy(VAUG[:, tb, 0:HD], pst[:])

            # ---- Phases 2+3 pools
            with (
                tc.tile_pool(name="big2", bufs=1) as big2,
                tc.tile_pool(name="esp", bufs=2) as esp,
                tc.tile_pool(name="work", bufs=4) as work,
                tc.tile_pool(name="owork", bufs=4) as owork,
                tc.tile_pool(name="ps_sc", bufs=2, space="PSUM") as ps_sc,
                tc.tile_pool(name="ps_y", bufs=2, space="PSUM") as ps_y,
                tc.tile_pool(name="ps_t", bufs=2, space="PSUM") as ps_t,
            ):
                YT = big2.tile([P, NHC, T], bf16, tag="YT")   # y^T per head [d, t]
                WOT = big2.tile([P, NHC, E], bf16, tag="WOT")
                for h in range(NHC):
                    nc.sync.dma_start(WOT[:, h], woT_r[:, h])

                def emit_oproj(tb, ec):
                    # One o_proj output block: out[tb, ec*512:+512] partial
                    # over this core's 4 heads. Shares the "psy" PSUM tag so
                    # phase-2 AV chains and o_proj groups rotate through the
                    # same two banks.
                    ps = ps_y.tile([P, 512], f32, tag="psy",
                                   name=f"po_{tb}_{ec}")
                    for h2 in range(NHC):
                        nc.tensor.matmul(
                            ps[:],
                            YT[:, h2, tb * P:(tb + 1) * P],
                            WOT[:, h2, ec * 512:(ec + 1) * 512],
                            start=(h2 == 0),
                            stop=(h2 == 3),
                        )
                    osb = owork.tile([P, 512], f16, tag="osb")
                    # Alternate copies between vector and scalar engines.
                    if ec % 2 == 0:
                        nc.vector.tensor_copy(osb[:], ps[:])
                    else:
                        nc.scalar.activation(osb[:], ps[:], COPY)
                    nc.sync.dma_start(
                        out_r[:, tb, ec * 512:(ec + 1) * 512], osb[:]
                    )

                # ---- Phase 2 (+ interleaved phase 3): causal attention,
                # transposed scores S^T[tk, tq]. Scores matmuls write 2-bank
                # pairs, exp'd in one scalar-engine call each; only the tk==tq
                # diagonal 128x128 sub-blocks need the triangular mask (on the
                # otherwise-idle gpsimd engine). The attn@V contraction runs
                # as one PSUM chain per output column block j over the es tile
                # that persists per (tqc, h). o_proj blocks of the previous
                # tqc are emitted between score pairs so the tensor engine
                # stays busy while the scalar engine works through the exps.
                for tqc in range(TQC):
                    ntk = 4 * (tqc + 1)   # tk blocks up to the diagonal
                    for h in range(NHC):
                        es = esp.tile([P, KO, 512], bf16, tag="es")
                        for pg in range(ntk // 2):
                            sc = ps_sc.tile([P, 2, 512], f32, tag="sc")
                            # Both blocks of the pair write from the pair's
                            # minimum causal offset so the exp'd range is
                            # fully covered; block bl=1's extra columns hold
                            # non-causal scores that no AV chain ever reads.
                            c0 = max(0, 2 * pg - 4 * tqc) * P
                            for bl in range(2):
                                tk = 2 * pg + bl
                                nc.tensor.matmul(
                                    sc[:, bl, c0:512],
                                    KT[:, tk * P:(tk + 1) * P],
                                    QT[:, h, tqc * 512 + c0:(tqc + 1) * 512],
                                    start=True,
                                    stop=True,
                                )
                            nc.scalar.activation(
                                es[:, 2 * pg:2 * pg + 2, c0:512],
                                sc[:, :, c0:512],
                                EXP,
                                scale=SCALE,
                            )
                            for bl in range(2):
                                tk = 2 * pg + bl
                                i = tk - 4 * tqc
                                if i >= 0:
                                    nc.gpsimd.tensor_mul(
                                        out=es[:, tk, i * P:(i + 1) * P],
                                        in0=es[:, tk, i * P:(i + 1) * P],
                                        in1=tri[:],
                                    )
                            if tqc >= 1 and pg >= ntk // 2 - 4:
                                emit_oproj(4 * (tqc - 1) + h, pg - (ntk // 2 - 4))
                        for j in range(4):
                            psy = ps_y.tile([P, 512], f32, tag="psy")
                            last = 4 * tqc + j
                            for tk in range(last + 1):
                                nc.tensor.matmul(
                                    psy[:, 0:HD + 1],
                                    es[:, tk, j * P:(j + 1) * P],
                                    VAUG[:, tk, 0:HD + 1],
                                    start=(tk == 0),
                                    stop=(tk == last),
                                )
                            jg = 4 * tqc + j
                            recip = work.tile([P, 1], f32, tag="recip")
                            nc.vector.reciprocal(recip[:], psy[:, HD:HD + 1])
                            ysb = work.tile([P, P], bf16, tag="ysb")
                            nc.vector.tensor_scalar_mul(ysb[:], psy[:, 0:HD], recip[:])
                            pst = ps_t.tile([P, P], bf16, tag="pst")
                            nc.tensor.transpose(pst[:], ysb[:], identity[:])
                            nc.vector.tensor_copy(YT[:, h, jg * P:(jg + 1) * P], pst[:])

                # Drain: o_proj blocks of the last tqc.
                for h in range(NHC):
                    for ec in range(4):
                        emit_oproj(12 + h, ec)

    nc.finalize()
    return nc


def _get_nc():
    if "nc" not in _NC_CACHE:
        _NC_CACHE["nc"] = _build_nc()
    return _NC_CACHE["nc"]


def _in_maps(x, wq, wk, wv, wo):
    import ml_dtypes

    bf = ml_dtypes.bfloat16
    xTb = [np.ascontiguousarray(x[b].T).astype(bf) for b in range(B)]
    wqT = [np.ascontiguousarray(wq[g * 512:(g + 1) * 512].T).astype(bf)
           for g in range(NKV)]
    wkT = [np.ascontiguousarray(wk[g * HD:(g + 1) * HD].T).astype(bf)
           for g in range(NKV)]
    wvT = [np.ascontiguousarray(wv[g * HD:(g + 1) * HD].T).astype(bf)
           for g in range(NKV)]
    woT = [np.ascontiguousarray(wo[:, g * 512:(g + 1) * 512].T).astype(bf)
           for g in range(NKV)]
    maps = []
    for c in range(N_CORES):
        b, g = divmod(c, NKV)
        maps.append({
            "xT": xTb[b],
            "wqT": wqT[g],
            "wkT": wkT[g],
            "wvT": wvT[g],
            "woT": woT[g],
        })
    return maps


def kernel(x, wq, wk, wv, wo):
    from concourse.bass_utils import run_bass_kernel_spmd

    x = np.asarray(x, dtype=np.float32)
    wq = np.asarray(wq, dtype=np.float32)
    wk = np.asarray(wk, dtype=np.float32)
    wv = np.asarray(wv, dtype=np.float32)
    wo = np.asarray(wo, dtype=np.float32)

    nc = _get_nc()
    in_maps = _in_maps(x, wq, wk, wv, wo)

    res = run_bass_kernel_spmd(nc, in_maps, core_ids=list(range(N_CORES)))

    partials = [res.results[c]["out"] for c in range(N_CORES)]
    out = np.empty((B, T, E), dtype=np.float32)
    for b in range(B):
        acc = partials[NKV * b].astype(np.float32)
        for g in range(1, NKV):
            acc = acc + partials[NKV * b + g]
        out[b] = acc
    return out


# revision 27
# speedup vs baseline: 58.1414x; 3.8801x over previous
"""GQA attention block (QKV proj + causal attention + output proj) on 8 trn2 cores.

Sharding: core c -> (batch b = c//4, kv-group g = c%4). Each core computes 4 Q
heads (one KV-head group) of one batch and a partial o_proj output; the host
sums the 4 partials per batch (row-sharded o_proj all-reduce done host-side).

All matmul operands are bf16 (fp32 PSUM accumulation) for 1-cycle/row tensor
throughput; the rel-err budget (2e-2 normalized absmax) dwarfs bf16 rounding.
Attention uses transposed scores S^T[tk, tq] so the softmax denominator comes
for free from a ones-column appended to V, and no on-chip transposes of
attention weights are needed. Scores are exp'd in 2-bank batches to amortize
the scalar engine's fixed per-instruction cost.
"""

import math

import numpy as np

# Model dims (hardcoded per contract; kernel.py must be self-contained).
B = 2
T = 2048
E = 2048
HD = 128               # head dim
NH = 16                # query heads total
NKV = 4                # kv heads total
NHC = 4                # query heads per core
P = 128
KO = E // P            # 16 contraction subtiles of 128
NQUART = 4             # xT streamed in 4 e-quarters of 512
TQC = T // 512         # 4 query chunks of 512
TB = T // P            # 16 t blocks of 128
SCALE = 1.0 / math.sqrt(HD)
N_CORES = 8

_NC_CACHE = {}


def _build_nc():
    import concourse.bacc as bacc
    import concourse.mybir as mybir
    import concourse.tile as tile
    from concourse.masks import make_identity, make_upper_triangular

    f32 = mybir.dt.float32
    bf16 = mybir.dt.bfloat16
    f16 = mybir.dt.float16
    EXP = mybir.ActivationFunctionType.Exp
    COPY = mybir.ActivationFunctionType.Copy
    nc = bacc.Bacc(None, target_bir_lowering=False)

    xT = nc.dram_tensor("xT", [E, T], bf16, kind="ExternalInput")
    wqT = nc.dram_tensor("wqT", [E, NHC * HD], bf16, kind="ExternalInput")
    wkT = nc.dram_tensor("wkT", [E, HD], bf16, kind="ExternalInput")
    wvT = nc.dram_tensor("wvT", [E, HD], bf16, kind="ExternalInput")
    woT = nc.dram_tensor("woT", [NHC * HD, E], bf16, kind="ExternalInput")
    # fp16 partials: halves the host-visible output transfer; 10-bit mantissa
    # rounding (~0.05%) is negligible vs the rel-err budget, and partial
    # magnitudes (~|out| <= ~30) sit comfortably in fp16 range.
    out = nc.dram_tensor("out", [T, E], f16, kind="ExternalOutput")

    xT_r = xT.rearrange("(ko p) t -> p ko t", p=P)        # [128, 16, T]
    wqT_r = wqT.rearrange("(ko p) d -> p ko d", p=P)      # [128, 16, 512]
    wkT_r = wkT.rearrange("(ko p) d -> p ko d", p=P)      # [128, 16, 128]
    wvT_r = wvT.rearrange("(ko p) d -> p ko d", p=P)
    woT_r = woT.rearrange("(h p) e -> p h e", p=P)        # [128, 4, E]
    out_r = out.rearrange("(tb p) e -> p tb e", p=P)      # [128, 16, E]

    with tile.TileContext(nc) as tc:
        with (
            tc.tile_pool(name="const", bufs=1) as constp,
            tc.tile_pool(name="qkv", bufs=1) as qkvp,
        ):
            identity = constp.tile([P, P], bf16, tag="ident")
            make_identity(nc, identity)

            # tri[p, q] = 1.0 where p <= q — causal mask for the one
            # tk==tq diagonal 128x128 sub-block.
            tri = constp.tile([P, P], bf16, tag="tri")
            make_upper_triangular(nc, tri[:], val=1.0, diag=True)

            QT = qkvp.tile([P, NHC, T], bf16, tag="QT")    # q^T per head [d, t]
            KT = qkvp.tile([P, T], bf16, tag="KT")         # k^T [d, t]
            VT = qkvp.tile([P, T], bf16, tag="VT")         # v^T [d, t]
            VAUG = qkvp.tile([P, TB, HD + 4], bf16, tag="VAUG")  # v blocks [tk, 129]

            # ---- Phase 1: projections. q^T/k^T/v^T = W @ x^T, contracting
            # over e; xT streamed in 4 e-quarters, weights resident. Quarter
            # partials accumulate in f32 SBUF; the last quarter also emits the
            # bf16 copy (on the scalar engine — the vector engine carries the
            # adds).
            with (
                tc.tile_pool(name="w1", bufs=1) as w1p,
                tc.tile_pool(name="acc1", bufs=1) as accp,
                tc.tile_pool(name="xq", bufs=2) as xqp,
                tc.tile_pool(name="ps1", bufs=4, space="PSUM") as ps1,
                tc.tile_pool(name="ps1t", bufs=2, space="PSUM") as ps1t,
            ):
                WQT = w1p.tile([P, KO, NHC * HD], bf16, tag="WQT")
                WKT = w1p.tile([P, KO, HD], bf16, tag="WKT")
                WVT = w1p.tile([P, KO, HD], bf16, tag="WVT")
                QTa = accp.tile([P, NHC, T], f32, tag="QTa")
                KTa = accp.tile([P, T], f32, tag="KTa")
                VTa = accp.tile([P, T], f32, tag="VTa")

                nc.vector.memset(VAUG[:, :, HD:HD + 1], 1.0)

                for q in range(NQUART):
                    xt = xqp.tile([P, 4, T], bf16, tag="xq")
                    # Interleave activations and weights per e-subtile so the
                    # first matmuls aren't stuck behind the full weight load.
                    if q == 0:
                        for eo in range(4):
                            nc.sync.dma_start(xt[:, eo], xT_r[:, eo])
                            nc.sync.dma_start(WQT[:, eo], wqT_r[:, eo])
                        nc.sync.dma_start(WKT[:, 0:4], wkT_r[:, 0:4])
                        nc.sync.dma_start(WVT[:, 0:4], wvT_r[:, 0:4])
                    else:
                        k4 = slice(4 * q, 4 * q + 4)
                        nc.sync.dma_start(xt[:], xT_r[:, k4])
                        nc.sync.dma_start(WQT[:, k4], wqT_r[:, k4])
                        nc.sync.dma_start(WKT[:, k4], wkT_r[:, k4])
                        nc.sync.dma_start(WVT[:, k4], wvT_r[:, k4])

                    def _acc(dsta, dstb, lhsT_of_eo, tcol):
                        ps = ps1.tile([P, 512], f32, tag="ps1")
                        for eo in range(4):
                            nc.tensor.matmul(
                                ps[:],
                                lhsT_of_eo(eo),
                                xt[:, eo, tcol * 512:(tcol + 1) * 512],
                                start=(eo == 0),
                                stop=(eo == 3),
                            )
                        if q == 0:
                            nc.vector.tensor_copy(dsta, ps[:])
                        else:
                            nc.vector.tensor_add(out=dsta, in0=dsta, in1=ps[:])
                        if q == 3:
                            nc.scalar.activation(dstb, dsta, COPY)

                    for h in range(NHC):
                        for tcol in range(TQC):
                            sl = slice(tcol * 512, (tcol + 1) * 512)
                            _acc(
                                QTa[:, h, sl],
                                QT[:, h, sl],
                                lambda eo, h=h: WQT[:, 4 * q + eo, h * HD:(h + 1) * HD],
                                tcol,
                            )
                    for tcol in range(TQC):
                        sl = slice(tcol * 512, (tcol + 1) * 512)
                        _acc(KTa[:, sl], KT[:, sl],
                             lambda eo: WKT[:, 4 * q + eo], tcol)
                    for tcol in range(TQC):
                        sl = slice(tcol * 512, (tcol + 1) * 512)
                        _acc(VTa[:, sl], VT[:, sl],
                             lambda eo: WVT[:, 4 * q + eo], tcol)
                        if q == 3:
                            # v^T -> v natural layout blocks feeding the
                            # ones-augmented V operand, as soon as each VT
                            # chunk is finalized.
                            for tb in range(4 * tcol, 4 * tcol + 4):
                                pst = ps1t.tile([P, P], bf16, tag="ps1t")
                                nc.tensor.transpose(
                                    pst[:], VT[:, tb * P:(tb + 1) * P], identity[:]
                                )
                                nc.vector.tensor_copy(VAUG[:, tb, 0:HD], pst[:])

            # ---- Phases 2+3 pools
            with (
                tc.tile_pool(name="big2", bufs=1) as big2,
                tc.tile_pool(name="esp", bufs=3) as esp,
                tc.tile_pool(name="work", bufs=8) as work,
                tc.tile_pool(name="owork", bufs=6) as owork,
                tc.tile_pool(name="ps_sc", bufs=2, space="PSUM") as ps_sc,
                tc.tile_pool(name="ps_y", bufs=2, space="PSUM") as ps_y,
                tc.tile_pool(name="ps_t", bufs=2, space="PSUM") as ps_t,
            ):
                YT = big2.tile([P, NHC, T], bf16, tag="YT")   # y^T per head [d, t]
                WOT = big2.tile([P, NHC, E], bf16, tag="WOT")
                for h in range(NHC):
                    nc.sync.dma_start(WOT[:, h], woT_r[:, h])

                osb_tiles = {}

                def emit_oproj(tb, ec, use_act=False):
                    # One o_proj output block: out[tb, ec*512:+512] partial
                    # over this core's 4 heads. Shares the "psy" PSUM tag so
                    # phase-2 AV chains and o_proj groups rotate through the
                    # same two banks. The 4 ec blocks of a tb stage into one
                    # [128, 2048] tile DMA'd out as a single 4KB-line
                    # transfer.
                    ps = ps_y.tile([P, 512], f32, tag="psy",
                                   name=f"po_{tb}_{ec}")
                    for h2 in range(NHC):
                        nc.tensor.matmul(
                            ps[:],
                            YT[:, h2, tb * P:(tb + 1) * P],
                            WOT[:, h2, ec * 512:(ec + 1) * 512],
                            start=(h2 == 0),
                            stop=(h2 == 3),
                        )
                    if tb not in osb_tiles:
                        osb_tiles[tb] = owork.tile(
                            [P, E], f16, tag="osb", name=f"osb_{tb}")
                    osb = osb_tiles[tb]
                    # The scalar engine is the contended one while exps
                    # stream, so interleaved blocks copy on the vector
                    # engine; the drain alternates both.
                    if use_act:
                        nc.scalar.activation(
                            osb[:, ec * 512:(ec + 1) * 512], ps[:], COPY)
                    else:
                        nc.vector.tensor_copy(
                            osb[:, ec * 512:(ec + 1) * 512], ps[:])
                    if tb == TB - 1:
                        # Last block: per-ec DMAs so the final transfer (and
                        # the teardown drain behind it) is 4x smaller.
                        nc.sync.dma_start(
                            out_r[:, tb, ec * 512:(ec + 1) * 512],
                            osb[:, ec * 512:(ec + 1) * 512])
                    elif ec == 3:
                        nc.sync.dma_start(out_r[:, tb], osb[:])

                # ---- Phase 2 (+ interleaved phase 3): causal attention,
                # transposed scores S^T[tk, tq]. Scores matmuls write 2-bank
                # pairs, exp'd in one scalar-engine call each; only the tk==tq
                # diagonal 128x128 sub-blocks need the triangular mask (on the
                # otherwise-idle gpsimd engine). The attn@V contraction runs
                # as one PSUM chain per output column block j over the es tile
                # that persists per (tqc, h). o_proj blocks of the previous
                # tqc are emitted between score pairs so the tensor engine
                # stays busy while the scalar engine works through the exps.
                for tqc in range(TQC):
                    ntk = 4 * (tqc + 1)   # tk blocks up to the diagonal
                    for h in range(NHC):
                        es = esp.tile([P, KO, 512], bf16, tag="es")
                        for pg in range(ntk // 2):
                            sc = ps_sc.tile([P, 2, 512], f32, tag="sc")
                            # Both blocks of the pair write from the pair's
                            # minimum causal offset so the exp'd range is
                            # fully covered; block bl=1's extra columns hold
                            # non-causal scores that no AV chain ever reads.
                            c0 = max(0, 2 * pg - 4 * tqc) * P
                            for bl in range(2):
                                tk = 2 * pg + bl
                                nc.tensor.matmul(
                                    sc[:, bl, c0:512],
                                    KT[:, tk * P:(tk + 1) * P],
                                    QT[:, h, tqc * 512 + c0:(tqc + 1) * 512],
                                    start=True,
                                    stop=True,
                                )
                            nc.scalar.activation(
                                es[:, 2 * pg:2 * pg + 2, c0:512],
                                sc[:, :, c0:512],
                                EXP,
                                scale=SCALE,
                            )
                            for bl in range(2):
                                tk = 2 * pg + bl
                                i = tk - 4 * tqc
                                if i >= 0:
                                    nc.gpsimd.tensor_mul(
                                        out=es[:, tk, i * P:(i + 1) * P],
                                        in0=es[:, tk, i * P:(i + 1) * P],
                                        in1=tri[:],
                                    )
                            if tqc >= 1 and pg >= ntk // 2 - 4:
                                emit_oproj(4 * (tqc - 1) + h, pg - (ntk // 2 - 4))
                        for j in range(4):
                            psy = ps_y.tile([P, 512], f32, tag="psy")
                            last = 4 * tqc + j
                            for tk in range(last + 1):
                                nc.tensor.matmul(
                                    psy[:, 0:HD + 1],
                                    es[:, tk, j * P:(j + 1) * P],
                                    VAUG[:, tk, 0:HD + 1],
                                    start=(tk == 0),
                                    stop=(tk == last),
                                )
                            jg = 4 * tqc + j
                            recip = work.tile([P, 1], f32, tag="recip")
                            nc.vector.reciprocal(recip[:], psy[:, HD:HD + 1])
                            ysb = work.tile([P, P], bf16, tag="ysb")
                            nc.vector.tensor_scalar_mul(ysb[:], psy[:, 0:HD], recip[:])
                            pst = ps_t.tile([P, P], bf16, tag="pst")
                            nc.tensor.transpose(pst[:], ysb[:], identity[:])
                            nc.vector.tensor_copy(YT[:, h, jg * P:(jg + 1) * P], pst[:])

                # Drain: o_proj blocks of the last tqc.
                for h in range(NHC):
                    for ec in range(4):
                        emit_oproj(12 + h, ec, use_act=(ec % 2 == 1))

    nc.finalize()
    return nc


def _get_nc():
    if "nc" not in _NC_CACHE:
        _NC_CACHE["nc"] = _build_nc()
    return _NC_CACHE["nc"]


def _in_maps(x, wq, wk, wv, wo):
    import ml_dtypes

    bf = ml_dtypes.bfloat16
    xTb = [np.ascontiguousarray(x[b].T).astype(bf) for b in range(B)]
    wqT = [np.ascontiguousarray(wq[g * 512:(g + 1) * 512].T).astype(bf)
           for g in range(NKV)]
    wkT = [np.ascontiguousarray(wk[g * HD:(g + 1) * HD].T).astype(bf)
           for g in range(NKV)]
    wvT = [np.ascontiguousarray(wv[g * HD:(g + 1) * HD].T).astype(bf)
           for g in range(NKV)]
    woT = [np.ascontiguousarray(wo[:, g * 512:(g + 1) * 512].T).astype(bf)
           for g in range(NKV)]
    maps = []
    for c in range(N_CORES):
        b, g = divmod(c, NKV)
        maps.append({
            "xT": xTb[b],
            "wqT": wqT[g],
            "wkT": wkT[g],
            "wvT": wvT[g],
            "woT": woT[g],
        })
    return maps


def kernel(x, wq, wk, wv, wo):
    from concourse.bass_utils import run_bass_kernel_spmd

    x = np.asarray(x, dtype=np.float32)
    wq = np.asarray(wq, dtype=np.float32)
    wk = np.asarray(wk, dtype=np.float32)
    wv = np.asarray(wv, dtype=np.float32)
    wo = np.asarray(wo, dtype=np.float32)

    nc = _get_nc()
    in_maps = _in_maps(x, wq, wk, wv, wo)

    res = run_bass_kernel_spmd(nc, in_maps, core_ids=list(range(N_CORES)))

    partials = [res.results[c]["out"] for c in range(N_CORES)]
    out = np.empty((B, T, E), dtype=np.float32)
    for b in range(B):
        acc = partials[NKV * b].astype(np.float32)
        for g in range(1, NKV):
            acc = acc + partials[NKV * b + g]
        out[b] = acc
    return out

